# revision 4
# baseline (speedup 1.0000x reference)
"""Trainium2 Bass kernel for nn_DipoleEnergyLean (gnn_message_passing), v2.

Changes vs baseline:
  - Edge gather uses the batched SWDGE dma_gather instruction (one
    instruction per 128-atom destination block gathers T*128 rows of
    256B) instead of one indirect_dma_start per 128-row tile. This
    removes ~1.6 ms of per-instruction SWDGE fixed overhead (994 ns x
    1584 instructions).
  - The neighbor-feature table is fp16 (256B rows): halves gather HBM
    traffic and AllGather size; edge matmuls run in fp16 (1 cyc/row vs
    4 for fp32).
  - The whole MLP/head runs in fp16 (1 cyc/row matmuls); x is uploaded
    pre-transposed and pre-cast to fp16 by the host, removing the
    on-device transpose pass.
  - AllGather writes the replicated table directly into Local DRAM (no
    Shared->Local bounce copy).
"""

import numpy as np

# ---------------------------------------------------------------- sizes
B, A, D, H, F = 256, 96, 384, 192, 96
S = 4
N = B * A                 # 24576 atoms
E = N * 32                # 786432 edges
CUTOFF = 5.2
NCORE = 8
AC = N // NCORE           # 3072 atoms / core
MOL = B // NCORE          # 32 molecules / core
import os as _os0
BLK = int(_os0.environ.get("BLKW", "64"))  # destination block (atoms)
NBLK = N // BLK           # global blocks
NBLK_C = AC // BLK        # blocks / core
NTA = AC // 128           # 24 atom tiles / core
FP = 128                  # padded feature row (256 B fp16) for gather
NCH = AC // 512           # 6 atom chunks of 512


# ---------------------------------------------------------------- host prep
def _preprocess_edges(atom_index12, distances, dp2, df2):
    """Sort contributions by destination; pad each 128-atom destination
    block to a uniform T tiles of 128 contributions (same T for all cores
    so one compiled program serves all 8). Emit gather indices in the
    int16 16-partition-wrapped layout dma_gather expects."""
    i0 = atom_index12[0].astype(np.int64)
    i1 = atom_index12[1].astype(np.int64)
    dest = np.concatenate([i0, i1])
    src = np.concatenate([i1, i0])
    dd = np.concatenate([distances, distances]).astype(np.float32)

    order = np.argsort(dest, kind="stable")
    dest_s = dest[order]
    src_s = src[order]
    dd_s = dd[order]

    counts = np.bincount(dest_s // BLK, minlength=NBLK)
    T = int(np.ceil(counts.max() / 128))
    NI = T * 128
    NI16 = NI // 16

    starts = np.zeros(NBLK + 1, np.int64)
    np.cumsum(counts, out=starts[1:])

    # decay weights on host: w = dp2 * exp(-df2*d) * smooth_cutoff(d)
    cx = np.clip((CUTOFF - dd_s) / CUTOFF, 0.0, 1.0).astype(np.float64)
    w_s = (dp2 * np.exp(-df2 * dd_s.astype(np.float64))
           * cx**3 * (cx * (6.0 * cx - 15.0) + 10.0)).astype(np.float32)

    KT = NBLK_C * T
    idxw = np.zeros((NCORE, 16, NBLK_C * NI16), np.int16)
    ohb = np.zeros((NCORE, 128, KT * BLK), np.float16)
    for c in range(NCORE):
        for bb in range(NBLK_C):
            g = c * NBLK_C + bb
            s0, s1 = starts[g], starts[g + 1]
            n = s1 - s0
            e = np.zeros(NI, np.int16)
            e[:n] = src_s[s0:s1].astype(np.int16)
            idxw[c, :, bb * NI16:(bb + 1) * NI16] = e.reshape(NI16, 16).T
            j = np.arange(n)
            t = j // 128
            p = j % 128
            d = (dest_s[s0:s1] - g * BLK).astype(np.int64)
            ohb[c, p, (bb * T + t) * BLK + d] = w_s[s0:s1].astype(np.float16)
    return T, idxw, ohb


# ---------------------------------------------------------------- device kernel
_CACHE = {}

# AllGather output mode: True = Shared scratchpad + bounce copy to Local
# (known-good baseline path), False = straight into Local DRAM.
import os as _os
_AG_SHARED = _os.environ.get("AG_SHARED", "0") == "1"
# edge-phase bisect knob: skip | one | q0 | full
_EDGE_MODE = _os.environ.get("EDGE_MODE", "full")
# oh-build engine split cycle: (DVE, Act, Pool) tiles out of every sum
_OH_SPLIT = tuple(int(v) for v in
                  _os.environ.get("OH_SPLIT", "2,1,0").split(","))


def _build(T, dp2, df2):
    import concourse.bass as bass
    import concourse.bacc as bacc
    import concourse.mybir as mybir
    import concourse.tile as tile
    from concourse.masks import make_identity

    f32 = mybir.dt.float32
    f16 = mybir.dt.float16
    i32 = mybir.dt.int32
    i16 = mybir.dt.int16
    AF = mybir.ActivationFunctionType
    OP = mybir.AluOpType
    NI = T * 128
    NI16 = NI // 16
    KT = NBLK_C * T

    nc = bacc.Bacc("TRN2", target_bir_lowering=False, num_devices=NCORE,
                   num_swdge_queues=4)

    xт_in = nc.dram_tensor("xT", [3, 128, AC], f16, kind="ExternalInput")
    sp_in = nc.dram_tensor("species", [1, AC], i32, kind="ExternalInput")
    tc_in = nc.dram_tensor("tcharge", [1, MOL], f32, kind="ExternalInput")
    w1_in = nc.dram_tensor("W1", [S, D, H], f16, kind="ExternalInput")
    w2_in = nc.dram_tensor("W2", [S, H, F], f16, kind="ExternalInput")
    wn_in = nc.dram_tensor("Wn", [S, F, F], f16, kind="ExternalInput")
    wf_in = nc.dram_tensor("Wf", [2, 96, S], f16, kind="ExternalInput")
    eidx_in = nc.dram_tensor("eidx", [16, NBLK_C * NI16], i16,
                             kind="ExternalInput")
    ohb_in = nc.dram_tensor("ohb", [128, KT * BLK], f16,
                            kind="ExternalInput")
    out_t = nc.dram_tensor("out", [2, AC], f32, kind="ExternalOutput")

    nbr_local = nc.dram_tensor("nbr_local", [AC, FP], f16)
    nbr_full = nc.dram_tensor("nbr_full", [N, FP], f16)
    nbr_full_sh = (nc.dram_tensor("nbr_full_sh", [N, FP], f16,
                                  addr_space="Shared")
                   if _AG_SHARED else None)

    with tile.TileContext(nc) as tc:
        # ---------------- persistent tiles (kept for the whole kernel)
        _keep = []

        def _single(shape, dtype, name):
            t, free = tc.tile(shape, dtype, name=name)
            _keep.append(free)
            return t

        ident16 = _single([128, 128], f16, "ident16")
        make_identity(nc, ident16[:, :])
        iota16 = _single([128, 128], f16, "iota16")
        internalT = _single([F, AC], f16, "internalT")
        mergedT = _single([F, AC], f16, "mergedT")
        eqs = _single([S, AC], f32, "eqs")
        spf16 = _single([1, AC], f16, "spf16")
        ones96 = _single([1, 96], f16, "ones96")
        nc.vector.memset(ones96[:, :], 1.0)

        ones4r = _single([1, S], f16, "ones4r")
        nc.vector.memset(ones4r[:, :], 1.0)
        with tc.tile_pool(name="init_pool", bufs=1) as ip, \
             tc.tile_pool(name="init_psum", bufs=2, space="PSUM") as ipp:
            iota_i = ip.tile([128, 128], i32, name="iota_i")
            nc.gpsimd.iota(iota_i[:, :], pattern=[[1, 128]], base=0,
                           channel_multiplier=0)
            nc.vector.tensor_copy(iota16[:, :], iota_i[:, :])
            sp_i = ip.tile([1, AC], i32, name="sp_i")
            nc.sync.dma_start(sp_i[:, :], sp_in[:, :])
            nc.vector.tensor_copy(spf16[:, :], sp_i[:, :])
            # eqs[s, a] = (species[a] == s): broadcast species to 4
            # partitions via K=1 matmul, compare against per-partition iota
            svec_i = ip.tile([S, 1], i32, name="svec_i")
            nc.gpsimd.iota(svec_i[:, :], pattern=[[0, 1]], base=0,
                           channel_multiplier=1)
            svec = ip.tile([S, 1], f32, name="svec")
            nc.vector.tensor_copy(svec[:, :], svec_i[:, :])
            for cch in range(NCH):
                sl = slice(cch * 512, (cch + 1) * 512)
                sp4 = ipp.tile([S, 512], f32, tag="sp4", name="sp4")
                nc.tensor.matmul(sp4[:, :], ones4r[:, :], spf16[:, sl],
                                 start=True, stop=True)
                nc.vector.tensor_scalar(
                    out=eqs[:, sl], in0=sp4[:, :], scalar1=svec[:, :],
                    scalar2=None, op0=OP.is_equal)

        tc.strict_bb_all_engine_barrier()

        # ---------------- phase 1: per-species MLP (transposed, fp16)
        with tc.tile_pool(name="mlp_sbuf", bufs=1) as mp, \
             tc.tile_pool(name="mlp_work", bufs=3) as wk:

            xT = [mp.tile([128, AC], f16, tag=f"xT{k}", name=f"xT{k}")
                  for k in range(3)]
            for k in range(3):
                nc.sync.dma_start(xT[k][:, :], xт_in[k, :, :])

            # weights
            w1t = [[mp.tile([128, H], f16, tag=f"w1_{s}_{k}", name=f"w1_{s}_{k}")
                    for k in range(3)] for s in range(S)]
            w2t = [[mp.tile([96, F], f16, tag=f"w2_{s}_{k}", name=f"w2_{s}_{k}")
                    for k in range(2)] for s in range(S)]
            wnt = [mp.tile([F, F], f16, tag=f"wn_{s}", name=f"wn_{s}")
                   for s in range(S)]
            for s in range(S):
                for k in range(3):
                    nc.sync.dma_start(w1t[s][k][:, :],
                                      w1_in[s, k * 128:(k + 1) * 128, :])
                for k in range(2):
                    nc.sync.dma_start(w2t[s][k][:, :],
                                      w2_in[s, k * 96:(k + 1) * 96, :])
                nc.sync.dma_start(wnt[s][:, :], wn_in[s, :, :])

            nbrT = mp.tile([F, AC], f16, tag="nbrT", name="nbrT")

            with tc.tile_pool(name="mlp_psum", bufs=2, space="PSUM") as pp, \
                 tc.tile_pool(name="mlp_psum1", bufs=1, space="PSUM") as pp1:
                for cch in range(NCH):
                    sl = slice(cch * 512, (cch + 1) * 512)
                    pn = pp1.tile([F, 512], f32, tag="pn", name="pn")
                    for s in range(S):
                        # species mask chunk, broadcast to 96 partitions
                        eq_c = wk.tile([1, 512], f16, tag="eq_c", name="eq_c")
                        nc.vector.tensor_scalar(
                            out=eq_c[:, :], in0=spf16[:, sl], scalar1=float(s),
                            scalar2=None, op0=OP.is_equal)
                        mps = pp.tile([F, 512], f32, tag="mps", name="mps")
                        nc.tensor.matmul(mps[:, :], ones96[:, :96], eq_c[:, :],
                                         start=True, stop=True)
                        mask_c = wk.tile([F, 512], f16, tag="mask_c",
                                         name="mask_c")
                        nc.vector.tensor_copy(mask_c[:, :], mps[:, :])

                        ph0 = pp.tile([96, 512], f32, tag="ph0", name="ph0")
                        ph1 = pp.tile([96, 512], f32, tag="ph1", name="ph1")
                        for k in range(3):
                            nc.tensor.matmul(ph0[:, :], w1t[s][k][:, 0:96],
                                             xT[k][:, sl], start=(k == 0),
                                             stop=(k == 2))
                        for k in range(3):
                            nc.tensor.matmul(ph1[:, :], w1t[s][k][:, 96:192],
                                             xT[k][:, sl], start=(k == 0),
                                             stop=(k == 2))
                        h0 = wk.tile([96, 512], f16, tag="h0", name="h0")
                        h1 = wk.tile([96, 512], f16, tag="h1", name="h1")
                        nc.scalar.activation(h0[:, :], ph0[:, :],
                                             AF.Gelu_apprx_tanh)
                        nc.scalar.activation(h1[:, :], ph1[:, :],
                                             AF.Gelu_apprx_tanh)
                        pv = pp1.tile([F, 512], f32, tag="pv", name="pv")
                        nc.tensor.matmul(pv[:, :], w2t[s][0][:, :], h0[:, :],
                                         start=True, stop=False)
                        nc.tensor.matmul(pv[:, :], w2t[s][1][:, :], h1[:, :],
                                         start=False, stop=True)
                        tmp = wk.tile([F, 512], f16, tag="tmp", name="tmp")
                        nc.vector.tensor_tensor(
                            out=tmp[:, :], in0=pv[:, :],
                            in1=mask_c[:, :], op=OP.mult)
                        if s == 0:
                            nc.vector.tensor_copy(internalT[:, sl], tmp[:, :])
                        else:
                            nc.vector.tensor_tensor(
                                out=internalT[:, sl], in0=internalT[:, sl],
                                in1=tmp[:, :], op=OP.add)
                        nc.tensor.matmul(pn[:, :], wnt[s][:, :], tmp[:, :],
                                         start=(s == 0), stop=(s == S - 1))
                    nc.vector.tensor_copy(nbrT[:, sl], pn[:, :])

            # stage nbr rows (transposed back) and write the local table
            with tc.tile_pool(name="st_psum", bufs=2, space="PSUM") as stp, \
                 tc.tile_pool(name="st_work", bufs=3) as stw:
                nbr_rows = nbr_local.rearrange("(t p) e -> p t e", p=128)
                for t in range(NTA):
                    ntp = stp.tile([128, 96], f16, tag="ntp", name="ntp")
                    nc.tensor.transpose(ntp[:, :],
                                        nbrT[:, t * 128:(t + 1) * 128],
                                        ident16[0:96, 0:96])
                    strow = stw.tile([128, FP], f16, tag="strow", name="strow")
                    nc.vector.tensor_copy(strow[:, 0:96], ntp[:, :])
                    nc.vector.memset(strow[:, 96:128], 0.0)
                    nc.sync.dma_start(nbr_rows[:, t, :], strow[:, :])

        tc.strict_bb_all_engine_barrier()

        # ---------------- phase 2+3: AllGather (on gpsimd) overlapped with
        # edge-phase prep (index loads + decay weights on DVE/Act/SP)
        with tc.tile_pool(name="edge_sbuf", bufs=1) as ep, \
             tc.tile_pool(name="edge_work", bufs=8) as ew, \
             tc.tile_pool(name="msg_pool", bufs=3) as mgp, \
             tc.tile_pool(name="edge_psum", bufs=2, space="PSUM") as epp, \
             tc.tile_pool(name="edge_psum2", bufs=2, space="PSUM") as epp2:

            # gather indices, replicated to all 8 gpsimd-core stripes
            idxw = ep.tile([128, NBLK_C * NI16], i16, tag="idxw", name="idxw")
            for kk in range(8):
                nc.sync.dma_start(idxw[16 * kk:16 * (kk + 1), :],
                                  eidx_in[:, :])
            # AllGather the nbr table (issued after prep so it overlaps)
            if _AG_SHARED:
                nc.gpsimd.collective_compute(
                    "AllGather", mybir.AluOpType.bypass,
                    replica_groups=[list(range(NCORE))],
                    ins=[nbr_local[:, :]],
                    outs=[nbr_full_sh[:, :]],
                )
                nc.sync.dma_start(nbr_full[:, :], nbr_full_sh[:, :])
            else:
                nc.gpsimd.collective_compute(
                    "AllGather", mybir.AluOpType.bypass,
                    replica_groups=[list(range(NCORE))],
                    ins=[nbr_local[:, :]],
                    outs=[nbr_full[:, :]],
                )

            nblocks = {"skip": 0, "one": 1}.get(_EDGE_MODE, NBLK_C)
            if nblocks < NBLK_C:
                nc.vector.memset(mergedT[:, :], 0.0)
            # oh-build engine split: cycle of DVE/Act/Pool assignments
            _D, _A, _P = _OH_SPLIT
            _CYC = _D + _A + _P
            CHUNK = 8            # tiles per gather (1024 idxs, packet limit)
            NCHG = (T + CHUNK - 1) // CHUNK
            qn = 0
            for b in range(nblocks):
                msgb = mgp.tile([128, NI], f16, tag="msg", name="msg")
                for g in range(NCHG):
                    t0 = g * CHUNK
                    nt = min(CHUNK, T - t0)
                    ni = nt * 128
                    nc.gpsimd.dma_gather(
                        out_ap=msgb[:, t0 * FP:(t0 + nt) * FP].rearrange(
                            "p (t e) -> p t e", e=FP),
                        in_ap=nbr_full[:, :],
                        idxs_ap=idxw[:, b * NI16 + t0 * 8:
                                     b * NI16 + (t0 + nt) * 8],
                        num_idxs=ni,
                        num_idxs_reg=ni,
                        elem_size=FP,
                        queue_num=qn % 4,
                    )
                    qn += 1
                pmT = epp.tile([96, BLK], f32, tag="pmT", name="pmT")
                for g0 in range(0, T, CHUNK):
                    nt0 = min(CHUNK, T - g0)
                    ohch = ew.tile([128, CHUNK * BLK], f16, tag="ohch",
                                   name="ohch")
                    nc.sync.dma_start(
                        ohch[:, :nt0 * BLK],
                        ohb_in[:, (b * T + g0) * BLK:
                               (b * T + g0 + nt0) * BLK])
                    for j in range(g0, g0 + nt0):
                        sl0 = (j - g0) * BLK
                        nc.tensor.matmul(pmT[:, :],
                                         msgb[:, j * FP:j * FP + 96],
                                         ohch[:, sl0:sl0 + BLK],
                                         start=(j == 0), stop=(j == T - 1))
                nc.vector.tensor_copy(mergedT[:, b * BLK:(b + 1) * BLK],
                                      pmT[:, :])

        tc.strict_bb_all_engine_barrier()

        # ---------------- phase 4: final head + charge redistribution
        with tc.tile_pool(name="head_sbuf", bufs=1) as hp, \
             tc.tile_pool(name="head_work", bufs=3) as hw, \
             tc.tile_pool(name="head_psum", bufs=2, space="PSUM") as hpp:

            wfi = hp.tile([96, S], f16, tag="wfi", name="wfi")
            wfm = hp.tile([96, S], f16, tag="wfm", name="wfm")
            nc.sync.dma_start(wfi[:, :], wf_in[0, :, :])
            nc.sync.dma_start(wfm[:, :], wf_in[1, :, :])

            pre = hp.tile([1, AC], f32, tag="pre", name="pre")
            chg = hp.tile([1, AC], f32, tag="chg", name="chg")
            ones4 = hp.tile([S, 1], f16, tag="ones4", name="ones4")
            nc.vector.memset(ones4[:, :], 1.0)

            for cch in range(NCH):
                sl = slice(cch * 512, (cch + 1) * 512)
                pp4 = hpp.tile([S, 512], f32, tag="pp4", name="pp4")
                nc.tensor.matmul(pp4[:, :], wfi[:, :], internalT[:, sl],
                                 start=True, stop=False)
                nc.tensor.matmul(pp4[:, :], wfm[:, :], mergedT[:, sl],
                                 start=False, stop=True)
                sel = hw.tile([S, 512], f16, tag="sel", name="sel")
                nc.vector.tensor_tensor(out=sel[:, :], in0=pp4[:, :],
                                        in1=eqs[:, sl], op=OP.mult)
                pr1 = hpp.tile([1, 512], f32, tag="pr1", name="pr1")
                nc.tensor.matmul(pr1[:, :], ones4[:, :], sel[:, :],
                                 start=True, stop=True)
                nc.vector.tensor_copy(pre[:, sl], pr1[:, :])

            # per-molecule redistribution
            tct = hp.tile([1, MOL], f32, tag="tct", name="tct")
            nc.sync.dma_start(tct[:, :], tc_in[:, :])
            msum = hp.tile([1, MOL], f32, tag="msum", name="msum")
            nc.vector.tensor_reduce(
                out=msum[:, :],
                in_=pre[:, :].rearrange("p (m a) -> p m a", a=A),
                axis=mybir.AxisListType.X, op=OP.add)
            adj = hp.tile([1, MOL], f32, tag="adj", name="adj")
            nc.vector.tensor_tensor(out=adj[:, :], in0=tct[:, :],
                                    in1=msum[:, :], op=OP.subtract)
            nc.vector.tensor_scalar(out=adj[:, :], in0=adj[:, :],
                                    scalar1=1.0 / A, scalar2=None, op0=OP.mult)
            nc.vector.tensor_tensor(
                out=chg[:, :].rearrange("p (m a) -> p m a", a=A),
                in0=pre[:, :].rearrange("p (m a) -> p m a", a=A),
                in1=adj[:, :].to_broadcast([1, MOL, A]),
                op=OP.add)

            nc.sync.dma_start(out_t[0:1, :], chg[:, :])
            nc.sync.dma_start(out_t[1:2, :], pre[:, :])

        for free in reversed(_keep):
            free()

    nc.compile()
    return nc


def _get_nc(T, dp2, df2):
    key = (T, round(float(dp2), 9), round(float(df2), 9))
    if key not in _CACHE:
        _CACHE[key] = _build(T, dp2, df2)
    return _CACHE[key]


# ---------------------------------------------------------------- entry point
def kernel(species, in_features, atom_index12, distances, total_charges,
           W1, W2, Wn, Wf, decay_prefactor, decay_factor, _trace=False):
    from concourse.bass_utils import run_bass_kernel_spmd

    species = np.asarray(species, np.int32)
    in_features = np.ascontiguousarray(np.asarray(in_features, np.float32))
    atom_index12 = np.asarray(atom_index12, np.int32)
    distances = np.asarray(distances, np.float32)
    total_charges = np.asarray(total_charges, np.float32)
    W1 = np.asarray(W1, np.float16)
    W2 = np.asarray(W2, np.float16)
    Wn = np.asarray(Wn, np.float16)
    Wf = np.asarray(Wf, np.float32)
    dp2 = float(np.asarray(decay_prefactor)) ** 2
    df2 = float(np.asarray(decay_factor)) ** 2

    # Wf [S, 2F, 1] -> [2, 96, S] fp16 (row 0: internal half, row 1: merged)
    wf2 = np.empty((2, 96, S), np.float16)
    for s in range(S):
        wf2[0, :, s] = Wf[s, 0:96, 0]
        wf2[1, :, s] = Wf[s, 96:192, 0]

    T, idxw, ohb = _preprocess_edges(atom_index12, distances, dp2, df2)
    nc = _get_nc(T, dp2, df2)

    in_maps = []
    for c in range(NCORE):
        xc = in_features[c * MOL:(c + 1) * MOL].reshape(AC, D)
        # xT[k][p, a] = x[a, 128k + p], fp16
        xt = np.ascontiguousarray(
            xc.reshape(AC, 3, 128).transpose(1, 2, 0)).astype(np.float16)
        in_maps.append({
            "xT": xt,
            "species": np.ascontiguousarray(
                species[c * MOL:(c + 1) * MOL].reshape(1, AC)),
            "tcharge": np.ascontiguousarray(
                total_charges[c * MOL:(c + 1) * MOL].reshape(1, MOL)),
            "W1": W1, "W2": W2, "Wn": Wn, "Wf": wf2,
            "eidx": np.ascontiguousarray(idxw[c]),
            "ohb": np.ascontiguousarray(ohb[c]),
        })

    res = run_bass_kernel_spmd(nc, in_maps, core_ids=list(range(NCORE)),
                               trace=_trace)
    charges = np.empty((B, A), np.float32)
    precharges = np.empty((B, A), np.float32)
    for c in range(NCORE):
        o = res.results[c]["out"]
        charges[c * MOL:(c + 1) * MOL] = o[0].reshape(MOL, A)
        precharges[c * MOL:(c + 1) * MOL] = o[1].reshape(MOL, A)
    if _trace:
        kernel._last_results = res
    return species.reshape(B, A), charges, precharges
